# revision 58
# baseline (speedup 1.0000x reference)
"""MultiHeadAttention (B=2, S=2048, D=2048, H=16, RoPE) on 8 NeuronCores.

Sharding: tensor-parallel over heads. Core c owns heads 2c, 2c+1 (256
channels). Each core: QKV projections for its channels, RoPE, attention for
its 2 heads, and a partial output projection y_c = ctx_c @ Wo[:, ch_c].T.
Host sums the 8 partials.

Datapath is bf16 into the PE (fp32 PSUM accumulation), tolerance 2e-2:
  - host pre-transposes x -> xT [D, B*S] bf16 and pre-arranges weights so
    every matmul operand has its contraction dim on partitions.
  - qT/kT produced in [head_dim, token] layout via PE; RoPE applied on DVE
    in bf16 with sign-folded half tables (no swap copies needed).
  - scores computed transposed: scrT[ktok, qtok] = kT_tile.T @ qT window;
    exp on ScalarE (scores ~N(0,1): no max subtraction needed) -> ex bf16.
  - PV uses ex tiles as the *stationary* operand so ctx lands in
    [qtok, ch] layout, and vS carries an extra all-ones column per head so
    the softmax denominator accumulates for free in the same matmuls.
  - normalization is then per-PARTITION: reciprocal + tensor_scalar on DVE.
  - ctx is transposed back to [ch, qtok] with the DMA xbar (idle engine)
    for the output projection; y goes PSUM -> HBM directly via DMA.
"""
import sys

sys.path.insert(0, "/opt/trn_rl_repo")

import numpy as np
import ml_dtypes

B, S, D, H = 2, 2048, 2048, 16
HD = D // H          # 128
HF = HD // 2         # 64
NCORES = 8
HPC = H // NCORES    # heads per core = 2
CPC = HPC * HD       # channels per core = 256
TOK = B * S          # 4096
P = 128
KT = D // P          # 16 contraction tiles
NCH = 256            # phase-1 token chunk
QC = 512             # phase-2 q chunk
SKT = S // P         # 16 key tiles per sequence
NCK = 512            # out-proj column chunk
ROPE_BASE = 10000.0

_cache = {}


def _build_nc():
    import concourse.bass as bass  # noqa: F401
    import concourse.mybir as mybir
    import concourse.tile as tile
    from concourse import bacc

    F32 = mybir.dt.float32
    BF16 = mybir.dt.bfloat16
    AF = mybir.ActivationFunctionType
    MUL = mybir.AluOpType.mult
    ADD = mybir.AluOpType.add

    nc = bacc.Bacc(None, target_bir_lowering=False)

    xT_d = nc.dram_tensor("xT", [D, TOK], BF16, kind="ExternalInput")
    wq_d = nc.dram_tensor("wqA", [P, KT, CPC], BF16, kind="ExternalInput")
    wk_d = nc.dram_tensor("wkA", [P, KT, CPC], BF16, kind="ExternalInput")
    wv_d = nc.dram_tensor("wvA", [P, KT, CPC], BF16, kind="ExternalInput")
    wo_d = nc.dram_tensor("woA", [P, HPC, D], BF16, kind="ExternalInput")
    cos_d = nc.dram_tensor("cosF", [P, S], BF16, kind="ExternalInput")
    sin_d = nc.dram_tensor("sinF", [P, S], BF16, kind="ExternalInput")
    y_d = nc.dram_tensor("y", [TOK, D], BF16, kind="ExternalOutput")

    SCALE = 1.0 / float(np.sqrt(HD))
    NQC = TOK // NCH            # 16 projection chunks
    SQC = S // NCH              # 8 chunks per sequence

    with tile.TileContext(nc) as tc, \
         nc.allow_low_precision(reason="bf16 datapath, tolerance 2e-2"):
        with tc.tile_pool(name="persist", bufs=1) as pers:
            wq = pers.tile([P, KT, CPC], BF16, name="wq")
            wk = pers.tile([P, KT, CPC], BF16, name="wk")
            wv = pers.tile([P, KT, CPC], BF16, name="wv")
            wo = pers.tile([P, HPC, D], BF16, name="wo")
            cosF = pers.tile([P, S], BF16, name="cosF")
            sinF = pers.tile([P, S], BF16, name="sinF")
            # weight/table loads ride the Activation HWDGE queue so they never
            # serialize against the xt stream on SP; split per kt-group so the
            # first matmuls wait only on the first slice
            # startup-critical slices on the Pool queue (it starts immediately;
            # the Act queue sits behind LoadActFuncSet), remainder split
            nc.gpsimd.dma_start(wq[:, 0:2, :], wq_d[:, 0:2, :])
            nc.gpsimd.dma_start(wk[:, 0:2, :], wk_d[:, 0:2, :])
            nc.gpsimd.dma_start(wv[:, 0:2, :], wv_d[:, 0:2, :])
            nc.scalar.dma_start(wq[:, 2:4, :], wq_d[:, 2:4, :])
            nc.scalar.dma_start(wk[:, 2:4, :], wk_d[:, 2:4, :])
            nc.scalar.dma_start(wv[:, 2:4, :], wv_d[:, 2:4, :])
            nc.gpsimd.dma_start(wq[:, 4:8, :], wq_d[:, 4:8, :])
            nc.gpsimd.dma_start(wk[:, 4:8, :], wk_d[:, 4:8, :])
            nc.gpsimd.dma_start(wv[:, 4:8, :], wv_d[:, 4:8, :])
            nc.scalar.dma_start(wq[:, 8:12, :], wq_d[:, 8:12, :])
            nc.scalar.dma_start(wk[:, 8:12, :], wk_d[:, 8:12, :])
            nc.scalar.dma_start(wv[:, 8:12, :], wv_d[:, 8:12, :])
            nc.gpsimd.dma_start(wq[:, 12:16, :], wq_d[:, 12:16, :])
            nc.gpsimd.dma_start(wk[:, 12:16, :], wk_d[:, 12:16, :])
            nc.scalar.dma_start(wv[:, 12:16, :], wv_d[:, 12:16, :])
            nc.scalar.dma_start(cosF[:], cos_d[:])
            nc.scalar.dma_start(sinF[:], sin_d[:])
            nc.scalar.dma_start(wo[:], wo_d[:])

            qT = [[pers.tile([P, S], BF16, name=f"qT{b}_{m}") for m in range(HPC)]
                  for b in range(B)]
            kT = [[pers.tile([P, S], BF16, name=f"kT{b}_{m}") for m in range(HPC)]
                  for b in range(B)]
            # v in [ktok, ch] layout + an all-ones column per head: the PV
            # matmuls then accumulate the softmax denominator for free.
            vS = [pers.tile([P, SKT, HPC, HD + 1], BF16, name=f"vS{b}")
                  for b in range(B)]
            for b in range(B):
                nc.vector.memset(vS[b][:, :, :, HD:HD + 1], 1.0)

            sb_pools = tc.tile_pool(name="xp", bufs=3), \
                tc.tile_pool(name="dp", bufs=4), \
                tc.tile_pool(name="rp", bufs=6), \
                tc.tile_pool(name="ep", bufs=2), \
                tc.tile_pool(name="cs", bufs=2), \
                tc.tile_pool(name="ct", bufs=2), \
                tc.tile_pool(name="rc", bufs=8), \
                tc.tile_pool(name="ys", bufs=3)
            xp, dp, rp, ep, csp, ctp, rcp, ysp = \
                [p.__enter__() for p in sb_pools]

            # ---------------- Phase 1: projections + RoPE ----------------
            with tc.tile_pool(name="pp1", bufs=2, space="PSUM") as pp1:
                for ch in range(NQC):
                    b, j = ch // SQC, ch % SQC
                    t0 = ch * NCH
                    s0 = j * NCH
                    # each [128, 2, 256] pair shares one PSUM bank (only the
                    # first matmul into the bank starts/zeroes it), so the
                    # whole chunk set is 3 banks and double-buffers in 6
                    q2 = pp1.tile([P, 2, NCH], F32, name="q2")
                    k2 = pp1.tile([P, 2, NCH], F32, name="k2")
                    v2 = pp1.tile([P, 2, CPC], F32, name="v2")
                    for kt in range(KT):
                        xt = xp.tile([P, NCH], BF16, name="xt")
                        nc.sync.dma_start(
                            xt[:], xT_d[kt * P:(kt + 1) * P, t0:t0 + NCH])
                        st_, sp_ = (kt == 0), (kt == KT - 1)
                        for m in range(HPC):
                            nc.tensor.matmul(
                                q2[:, m, :], wq[:, kt, m * P:(m + 1) * P],
                                xt[:], start=st_ and m == 0,
                                stop=sp_ and m == 1, skip_group_check=True)
                            nc.tensor.matmul(
                                k2[:, m, :], wk[:, kt, m * P:(m + 1) * P],
                                xt[:], start=st_ and m == 0,
                                stop=sp_ and m == 1, skip_group_check=True)
                        for st in range(2):
                            nc.tensor.matmul(
                                v2[:, st, :], xt[:, st * P:(st + 1) * P],
                                wv[:, kt, :], start=st_ and st == 0,
                                stop=sp_ and st == 1, skip_group_check=True)
                    for st in range(2):
                        g = j * 2 + st
                        # v drain first in the DVE stream (GPSIMD cannot read
                        # PSUM) so the v PSUM banks free up promptly
                        nc.vector.tensor_copy(
                            vS[b][:, g, 0, 0:HD], v2[:, st, 0:HD])
                        nc.vector.tensor_copy(
                            vS[b][:, g, 1, 0:HD], v2[:, st, HD:2 * HD])
                    # RoPE: dst = q*cosF + t1, t1 built from sign-folded sinF
                    for i, dst in enumerate(
                            [qT[b][0], qT[b][1], kT[b][0], kT[b][1]]):
                        sb = dp.tile([P, NCH], BF16, name="sb")
                        nc.scalar.copy(sb[:], (q2 if i < 2 else k2)[:, i % 2, :])
                        t1 = rp.tile([P, NCH], BF16, name="t1")
                        t2 = rp.tile([P, NCH], BF16, name="t2")
                        nc.vector.tensor_tensor(
                            t1[0:HF, :], sb[HF:P, :],
                            sinF[HF:P, s0:s0 + NCH], MUL)
                        nc.vector.tensor_tensor(
                            t1[HF:P, :], sb[0:HF, :],
                            sinF[0:HF, s0:s0 + NCH], MUL)
                        nc.vector.tensor_tensor(
                            t2[:], sb[:], cosF[:, s0:s0 + NCH], MUL)
                        nc.vector.tensor_tensor(
                            dst[:, s0:s0 + NCH], t2[:], t1[:], ADD)


            # ------------- Phase 2: attention + output projection ---------
            with tc.tile_pool(name="sp2", bufs=4, space="PSUM") as spsum, \
                 tc.tile_pool(name="cp", bufs=1, space="PSUM") as cpsum, \
                 tc.tile_pool(name="yp", bufs=2, space="PSUM") as ypsum:
                def outproj_unit(pend, u, tail=False):
                    # one (tt, nck) unit of the *previous* q-chunk's output
                    # projection, interleaved into the current chunk's PE
                    # stream to fill exp-wait gaps
                    row0, pctxT = pend
                    tt, nck = u // 4, u % 4
                    y_ps = ypsum.tile([P, NCK], F32, name="y_ps")
                    for m in range(HPC):
                        nc.tensor.matmul(
                            y_ps[:],
                            pctxT[m][:, tt * P:(tt + 1) * P],
                            wo[:, m, nck * NCK:(nck + 1) * NCK],
                            start=(m == 0), stop=(m == HPC - 1))
                    y_sb = ysp.tile([P, NCK], BF16, name="y_sb")
                    nc.vector.tensor_copy(y_sb[:], y_ps[:])
                    nc.sync.dma_start(
                        y_d[row0 + tt * P:row0 + (tt + 1) * P,
                            nck * NCK:(nck + 1) * NCK],
                        y_sb[:])

                pending = None
                for b in range(B):
                    for qc in range(S // QC):
                        q0 = qc * QC
                        ctxT = [ctp.tile([P, QC], BF16, name=f"ctxT{m}")
                                for m in range(HPC)]
                        ctx_sb = [csp.tile([P, CPC], BF16, name=f"cs{qt}")
                                  for qt in range(QC // P)]
                        for m in range(HPC):
                            ex = ep.tile([P, SKT, QC], BF16, name="ex")
                            # two q-tile accumulators share one PSUM bank:
                            # only the very first matmul into the bank starts
                            # (zeroes) it, everything else accumulates.
                            ctx2 = [cpsum.tile([P, 2, HD + 1], F32,
                                               name=f"c2_{j}")
                                    for j in range(QC // P // 2)]
                            for kt2 in range(SKT // 2):
                                for i in range(2):
                                    scr1 = spsum.tile([P, QC], F32, name="scr1")
                                    nc.tensor.matmul(
                                        scr1[:],
                                        kT[b][m][:, (2 * kt2 + i) * P:
                                                 (2 * kt2 + i + 1) * P],
                                        qT[b][m][:, q0:q0 + QC],
                                        start=True, stop=True)
                                    nc.scalar.activation(
                                        ex[:, 2 * kt2 + i, :], scr1[:],
                                        AF.Exp, scale=SCALE)
                                for i in range(2):
                                    kt = 2 * kt2 + i
                                    for qt in range(QC // P):
                                        nc.tensor.matmul(
                                            ctx2[qt // 2][:, qt % 2, :],
                                            ex[:, kt, qt * P:(qt + 1) * P],
                                            vS[b][:, kt, m, :],
                                            start=(kt == 0 and qt % 2 == 0),
                                            stop=(kt == SKT - 1 and qt % 2 == 1),
                                            skip_group_check=True)
                                if pending is not None:
                                    outproj_unit(pending, m * 8 + kt2)
                            for qt in range(QC // P):
                                rec = rcp.tile([P, 1], F32, name="rec")
                                nc.vector.reciprocal(
                                    rec[:], ctx2[qt // 2][:, qt % 2, HD:HD + 1])
                                nc.vector.tensor_scalar(
                                    out=ctx_sb[qt][:, m * HD:(m + 1) * HD],
                                    in0=ctx2[qt // 2][:, qt % 2, 0:HD],
                                    scalar1=rec[:], scalar2=None, op0=MUL)
                                nc.sync.dma_start(
                                    ctxT[m][:, qt * P:(qt + 1) * P],
                                    ctx_sb[qt][:, m * HD:(m + 1) * HD],
                                    transpose=True)
                        pending = (b * S + q0, ctxT)
                for u in range(16):
                    outproj_unit(pending, u, tail=True)
            for p in reversed(sb_pools):
                p.__exit__(None, None, None)
    nc.finalize()
    return nc


def _rope_tables():
    inv_freq = (1.0 / (ROPE_BASE ** (np.arange(0, HD, 2, dtype=np.float32) / HD))).astype(np.float32)
    t = np.arange(S, dtype=np.float32)
    freqs = np.outer(t, inv_freq).astype(np.float32)  # [S, HD/2]
    c = freqs.T.copy()
    cosF = np.concatenate([np.cos(c), np.cos(c)], axis=0)    # [128, S]
    sinF = np.concatenate([np.sin(c), -np.sin(c)], axis=0)   # [128, S]
    bf = ml_dtypes.bfloat16
    return np.ascontiguousarray(cosF.astype(bf)), np.ascontiguousarray(sinF.astype(bf))


def kernel(x, Wq, Wk, Wv, Wo):
    from concourse.bass_utils import run_bass_kernel_spmd

    bf = ml_dtypes.bfloat16
    x = np.asarray(x, dtype=np.float32)
    Wq = np.asarray(Wq, dtype=np.float32)
    Wk = np.asarray(Wk, dtype=np.float32)
    Wv = np.asarray(Wv, dtype=np.float32)
    Wo = np.asarray(Wo, dtype=np.float32)

    xT = np.ascontiguousarray(x.reshape(TOK, D).T.astype(bf))  # [D, TOK]
    cosF, sinF = _rope_tables()

    def warr(W, c0, c1):
        # [P, KT, CPC]: warr[p, kt, c] = W[c0+c, kt*128+p]
        Wc = W[c0:c1, :]                      # [CPC, D]
        arr = Wc.T.reshape(KT, P, CPC).transpose(1, 0, 2)
        return np.ascontiguousarray(arr.astype(bf))

    in_maps = []
    for c in range(NCORES):
        ch0, ch1 = c * CPC, (c + 1) * CPC
        Woc = Wo[:, ch0:ch1]                  # [D, CPC]
        woA = Woc.T.reshape(HPC, P, D).transpose(1, 0, 2)
        in_maps.append({
            "xT": xT,
            "wqA": warr(Wq, ch0, ch1),
            "wkA": warr(Wk, ch0, ch1),
            "wvA": warr(Wv, ch0, ch1),
            "woA": np.ascontiguousarray(woA.astype(bf)),
            "cosF": cosF,
            "sinF": sinF,
        })

    if "nc" not in _cache:
        _cache["nc"] = _build_nc()
    res = run_bass_kernel_spmd(_cache["nc"], in_maps, core_ids=list(range(NCORES)))
    _cache["last_results"] = res

    y = np.zeros((TOK, D), dtype=np.float32)
    for rm in res.results:
        y += rm["y"].astype(np.float32)
    return y.reshape(B, S, D)


# revision 59
# speedup vs baseline: 1.0075x; 1.0075x over previous
"""MultiHeadAttention (B=2, S=2048, D=2048, H=16, RoPE) on 8 NeuronCores.

Sharding: tensor-parallel over heads. Core c owns heads 2c, 2c+1 (256
channels). Each core: QKV projections for its channels, RoPE, attention for
its 2 heads, and a partial output projection y_c = ctx_c @ Wo[:, ch_c].T.
Host sums the 8 partials.

Datapath is bf16 into the PE (fp32 PSUM accumulation), tolerance 2e-2:
  - host pre-transposes x -> xT [D, B*S] bf16 and pre-arranges weights so
    every matmul operand has its contraction dim on partitions.
  - qT/kT produced in [head_dim, token] layout via PE; RoPE applied on DVE
    in bf16 with sign-folded half tables (no swap copies needed).
  - scores computed transposed: scrT[ktok, qtok] = kT_tile.T @ qT window;
    exp on ScalarE (scores ~N(0,1): no max subtraction needed) -> ex bf16.
  - PV uses ex tiles as the *stationary* operand so ctx lands in
    [qtok, ch] layout, and vS carries an extra all-ones column per head so
    the softmax denominator accumulates for free in the same matmuls.
  - normalization is then per-PARTITION: reciprocal + tensor_scalar on DVE.
  - ctx is transposed back to [ch, qtok] with the DMA xbar (idle engine)
    for the output projection; y goes PSUM -> HBM directly via DMA.
"""
import sys

sys.path.insert(0, "/opt/trn_rl_repo")

import numpy as np
import ml_dtypes

B, S, D, H = 2, 2048, 2048, 16
HD = D // H          # 128
HF = HD // 2         # 64
NCORES = 8
HPC = H // NCORES    # heads per core = 2
CPC = HPC * HD       # channels per core = 256
TOK = B * S          # 4096
P = 128
KT = D // P          # 16 contraction tiles
NCH = 256            # phase-1 token chunk
QC = 512             # phase-2 q chunk
SKT = S // P         # 16 key tiles per sequence
NCK = 512            # out-proj column chunk
ROPE_BASE = 10000.0

_cache = {}


def _build_nc():
    import concourse.bass as bass  # noqa: F401
    import concourse.mybir as mybir
    import concourse.tile as tile
    from concourse import bacc

    F32 = mybir.dt.float32
    BF16 = mybir.dt.bfloat16
    AF = mybir.ActivationFunctionType
    MUL = mybir.AluOpType.mult
    ADD = mybir.AluOpType.add

    nc = bacc.Bacc(None, target_bir_lowering=False)

    xT_d = nc.dram_tensor("xT", [D, TOK], BF16, kind="ExternalInput")
    wq_d = nc.dram_tensor("wqA", [P, KT, CPC], BF16, kind="ExternalInput")
    wk_d = nc.dram_tensor("wkA", [P, KT, CPC], BF16, kind="ExternalInput")
    wv_d = nc.dram_tensor("wvA", [P, KT, CPC], BF16, kind="ExternalInput")
    wo_d = nc.dram_tensor("woA", [P, HPC, D], BF16, kind="ExternalInput")
    cos_d = nc.dram_tensor("cosF", [P, S], BF16, kind="ExternalInput")
    sin_d = nc.dram_tensor("sinF", [P, S], BF16, kind="ExternalInput")
    y_d = nc.dram_tensor("y", [TOK, D], BF16, kind="ExternalOutput")

    SCALE = 1.0 / float(np.sqrt(HD))
    NQC = TOK // NCH            # 16 projection chunks
    SQC = S // NCH              # 8 chunks per sequence

    with tile.TileContext(nc) as tc, \
         nc.allow_low_precision(reason="bf16 datapath, tolerance 2e-2"):
        with tc.tile_pool(name="persist", bufs=1) as pers:
            wq = pers.tile([P, KT, CPC], BF16, name="wq")
            wk = pers.tile([P, KT, CPC], BF16, name="wk")
            wv = pers.tile([P, KT, CPC], BF16, name="wv")
            wo = pers.tile([P, HPC, D], BF16, name="wo")
            cosF = pers.tile([P, S], BF16, name="cosF")
            sinF = pers.tile([P, S], BF16, name="sinF")
            # weight/table loads ride the Activation HWDGE queue so they never
            # serialize against the xt stream on SP; split per kt-group so the
            # first matmuls wait only on the first slice
            # startup-critical slices on the Pool queue (it starts immediately;
            # the Act queue sits behind LoadActFuncSet), remainder split
            nc.gpsimd.dma_start(wq[:, 0:2, :], wq_d[:, 0:2, :])
            nc.gpsimd.dma_start(wk[:, 0:2, :], wk_d[:, 0:2, :])
            nc.gpsimd.dma_start(wv[:, 0:2, :], wv_d[:, 0:2, :])
            nc.scalar.dma_start(wq[:, 2:4, :], wq_d[:, 2:4, :])
            nc.scalar.dma_start(wk[:, 2:4, :], wk_d[:, 2:4, :])
            nc.scalar.dma_start(wv[:, 2:4, :], wv_d[:, 2:4, :])
            nc.gpsimd.dma_start(wq[:, 4:8, :], wq_d[:, 4:8, :])
            nc.gpsimd.dma_start(wk[:, 4:8, :], wk_d[:, 4:8, :])
            nc.gpsimd.dma_start(wv[:, 4:8, :], wv_d[:, 4:8, :])
            nc.scalar.dma_start(wq[:, 8:12, :], wq_d[:, 8:12, :])
            nc.scalar.dma_start(wk[:, 8:12, :], wk_d[:, 8:12, :])
            nc.scalar.dma_start(wv[:, 8:12, :], wv_d[:, 8:12, :])
            nc.gpsimd.dma_start(wq[:, 12:16, :], wq_d[:, 12:16, :])
            nc.gpsimd.dma_start(wk[:, 12:16, :], wk_d[:, 12:16, :])
            nc.scalar.dma_start(wv[:, 12:16, :], wv_d[:, 12:16, :])
            nc.scalar.dma_start(cosF[:], cos_d[:])
            nc.scalar.dma_start(sinF[:], sin_d[:])
            nc.scalar.dma_start(wo[:], wo_d[:])

            qT = [[pers.tile([P, S], BF16, name=f"qT{b}_{m}") for m in range(HPC)]
                  for b in range(B)]
            kT = [[pers.tile([P, S], BF16, name=f"kT{b}_{m}") for m in range(HPC)]
                  for b in range(B)]
            # v in [ktok, ch] layout + an all-ones column per head: the PV
            # matmuls then accumulate the softmax denominator for free.
            vS = [pers.tile([P, SKT, HPC, HD + 1], BF16, name=f"vS{b}")
                  for b in range(B)]
            for b in range(B):
                nc.vector.memset(vS[b][:, :, :, HD:HD + 1], 1.0)

            sb_pools = tc.tile_pool(name="xp", bufs=3), \
                tc.tile_pool(name="dp", bufs=4), \
                tc.tile_pool(name="rp", bufs=6), \
                tc.tile_pool(name="ep", bufs=2), \
                tc.tile_pool(name="cs", bufs=2), \
                tc.tile_pool(name="ct", bufs=2), \
                tc.tile_pool(name="rc", bufs=8), \
                tc.tile_pool(name="ys", bufs=3)
            xp, dp, rp, ep, csp, ctp, rcp, ysp = \
                [p.__enter__() for p in sb_pools]

            # ---------------- Phase 1: projections + RoPE ----------------
            with tc.tile_pool(name="pp1", bufs=2, space="PSUM") as pp1:
                for ch in range(NQC):
                    b, j = ch // SQC, ch % SQC
                    t0 = ch * NCH
                    s0 = j * NCH
                    # each [128, 2, 256] pair shares one PSUM bank (only the
                    # first matmul into the bank starts/zeroes it), so the
                    # whole chunk set is 3 banks and double-buffers in 6
                    q2 = pp1.tile([P, 2, NCH], F32, name="q2")
                    k2 = pp1.tile([P, 2, NCH], F32, name="k2")
                    v2 = pp1.tile([P, 2, CPC], F32, name="v2")
                    for kt in range(KT):
                        xt = xp.tile([P, NCH], BF16, name="xt")
                        nc.sync.dma_start(
                            xt[:], xT_d[kt * P:(kt + 1) * P, t0:t0 + NCH])
                        st_, sp_ = (kt == 0), (kt == KT - 1)
                        for m in range(HPC):
                            nc.tensor.matmul(
                                q2[:, m, :], wq[:, kt, m * P:(m + 1) * P],
                                xt[:], start=st_ and m == 0,
                                stop=sp_ and m == 1, skip_group_check=True)
                            nc.tensor.matmul(
                                k2[:, m, :], wk[:, kt, m * P:(m + 1) * P],
                                xt[:], start=st_ and m == 0,
                                stop=sp_ and m == 1, skip_group_check=True)
                        for st in range(2):
                            nc.tensor.matmul(
                                v2[:, st, :], xt[:, st * P:(st + 1) * P],
                                wv[:, kt, :], start=st_ and st == 0,
                                stop=sp_ and st == 1, skip_group_check=True)
                    for st in range(2):
                        g = j * 2 + st
                        # v drain first in the DVE stream (GPSIMD cannot read
                        # PSUM) so the v PSUM banks free up promptly
                        nc.vector.tensor_copy(
                            vS[b][:, g, 0, 0:HD], v2[:, st, 0:HD])
                        nc.vector.tensor_copy(
                            vS[b][:, g, 1, 0:HD], v2[:, st, HD:2 * HD])
                    # RoPE: dst = q*cosF + t1, t1 built from sign-folded sinF
                    for i, dst in enumerate(
                            [qT[b][0], qT[b][1], kT[b][0], kT[b][1]]):
                        sb = dp.tile([P, NCH], BF16, name="sb")
                        nc.scalar.copy(sb[:], (q2 if i < 2 else k2)[:, i % 2, :])
                        t1 = rp.tile([P, NCH], BF16, name="t1")
                        t2 = rp.tile([P, NCH], BF16, name="t2")
                        nc.vector.tensor_tensor(
                            t1[0:HF, :], sb[HF:P, :],
                            sinF[HF:P, s0:s0 + NCH], MUL)
                        nc.vector.tensor_tensor(
                            t1[HF:P, :], sb[0:HF, :],
                            sinF[0:HF, s0:s0 + NCH], MUL)
                        nc.vector.tensor_tensor(
                            t2[:], sb[:], cosF[:, s0:s0 + NCH], MUL)
                        nc.vector.tensor_tensor(
                            dst[:, s0:s0 + NCH], t2[:], t1[:], ADD)


            # ------------- Phase 2: attention + output projection ---------
            with tc.tile_pool(name="sp2", bufs=2, space="PSUM") as spsum, \
                 tc.tile_pool(name="cp", bufs=1, space="PSUM") as cpsum, \
                 tc.tile_pool(name="yp", bufs=2, space="PSUM") as ypsum:
                def outproj_unit(pend, u, tail=False):
                    # one (tt, nck) unit of the *previous* q-chunk's output
                    # projection, interleaved into the current chunk's PE
                    # stream to fill exp-wait gaps
                    row0, pctxT = pend
                    tt, nck = u // 4, u % 4
                    y_ps = ypsum.tile([P, NCK], F32, name="y_ps")
                    for m in range(HPC):
                        nc.tensor.matmul(
                            y_ps[:],
                            pctxT[m][:, tt * P:(tt + 1) * P],
                            wo[:, m, nck * NCK:(nck + 1) * NCK],
                            start=(m == 0), stop=(m == HPC - 1))
                    y_sb = ysp.tile([P, NCK], BF16, name="y_sb")
                    nc.vector.tensor_copy(y_sb[:], y_ps[:])
                    nc.sync.dma_start(
                        y_d[row0 + tt * P:row0 + (tt + 1) * P,
                            nck * NCK:(nck + 1) * NCK],
                        y_sb[:])

                pending = None
                for b in range(B):
                    for qc in range(S // QC):
                        q0 = qc * QC
                        ctxT = [ctp.tile([P, QC], BF16, name=f"ctxT{m}")
                                for m in range(HPC)]
                        ctx_sb = [csp.tile([P, CPC], BF16, name=f"cs{qt}")
                                  for qt in range(QC // P)]
                        for m in range(HPC):
                            ex = ep.tile([P, SKT, QC], BF16, name="ex")
                            # two q-tile accumulators share one PSUM bank:
                            # only the very first matmul into the bank starts
                            # (zeroes) it, everything else accumulates.
                            ctx2 = [cpsum.tile([P, 2, HD + 1], F32,
                                               name=f"c2_{j}")
                                    for j in range(QC // P // 2)]
                            for kt2 in range(SKT // 2):
                                scr2 = spsum.tile([P, 2, QC], F32, name="scr2")
                                for i in range(2):
                                    nc.tensor.matmul(
                                        scr2[:, i, :],
                                        kT[b][m][:, (2 * kt2 + i) * P:
                                                 (2 * kt2 + i + 1) * P],
                                        qT[b][m][:, q0:q0 + QC],
                                        start=True, stop=True)
                                nc.scalar.activation(
                                    ex[:, 2 * kt2:2 * kt2 + 2, :], scr2[:],
                                    AF.Exp, scale=SCALE)
                                for i in range(2):
                                    kt = 2 * kt2 + i
                                    for qt in range(QC // P):
                                        nc.tensor.matmul(
                                            ctx2[qt // 2][:, qt % 2, :],
                                            ex[:, kt, qt * P:(qt + 1) * P],
                                            vS[b][:, kt, m, :],
                                            start=(kt == 0 and qt % 2 == 0),
                                            stop=(kt == SKT - 1 and qt % 2 == 1),
                                            skip_group_check=True)
                                if pending is not None:
                                    outproj_unit(pending, m * 8 + kt2)
                            for qt in range(QC // P):
                                rec = rcp.tile([P, 1], F32, name="rec")
                                nc.vector.reciprocal(
                                    rec[:], ctx2[qt // 2][:, qt % 2, HD:HD + 1])
                                nc.vector.tensor_scalar(
                                    out=ctx_sb[qt][:, m * HD:(m + 1) * HD],
                                    in0=ctx2[qt // 2][:, qt % 2, 0:HD],
                                    scalar1=rec[:], scalar2=None, op0=MUL)
                                nc.sync.dma_start(
                                    ctxT[m][:, qt * P:(qt + 1) * P],
                                    ctx_sb[qt][:, m * HD:(m + 1) * HD],
                                    transpose=True)
                        pending = (b * S + q0, ctxT)
                for u in range(16):
                    outproj_unit(pending, u, tail=True)
            for p in reversed(sb_pools):
                p.__exit__(None, None, None)
    nc.finalize()
    return nc


def _rope_tables():
    inv_freq = (1.0 / (ROPE_BASE ** (np.arange(0, HD, 2, dtype=np.float32) / HD))).astype(np.float32)
    t = np.arange(S, dtype=np.float32)
    freqs = np.outer(t, inv_freq).astype(np.float32)  # [S, HD/2]
    c = freqs.T.copy()
    cosF = np.concatenate([np.cos(c), np.cos(c)], axis=0)    # [128, S]
    sinF = np.concatenate([np.sin(c), -np.sin(c)], axis=0)   # [128, S]
    bf = ml_dtypes.bfloat16
    return np.ascontiguousarray(cosF.astype(bf)), np.ascontiguousarray(sinF.astype(bf))


def kernel(x, Wq, Wk, Wv, Wo):
    from concourse.bass_utils import run_bass_kernel_spmd

    bf = ml_dtypes.bfloat16
    x = np.asarray(x, dtype=np.float32)
    Wq = np.asarray(Wq, dtype=np.float32)
    Wk = np.asarray(Wk, dtype=np.float32)
    Wv = np.asarray(Wv, dtype=np.float32)
    Wo = np.asarray(Wo, dtype=np.float32)

    xT = np.ascontiguousarray(x.reshape(TOK, D).T.astype(bf))  # [D, TOK]
    cosF, sinF = _rope_tables()

    def warr(W, c0, c1):
        # [P, KT, CPC]: warr[p, kt, c] = W[c0+c, kt*128+p]
        Wc = W[c0:c1, :]                      # [CPC, D]
        arr = Wc.T.reshape(KT, P, CPC).transpose(1, 0, 2)
        return np.ascontiguousarray(arr.astype(bf))

    in_maps = []
    for c in range(NCORES):
        ch0, ch1 = c * CPC, (c + 1) * CPC
        Woc = Wo[:, ch0:ch1]                  # [D, CPC]
        woA = Woc.T.reshape(HPC, P, D).transpose(1, 0, 2)
        in_maps.append({
            "xT": xT,
            "wqA": warr(Wq, ch0, ch1),
            "wkA": warr(Wk, ch0, ch1),
            "wvA": warr(Wv, ch0, ch1),
            "woA": np.ascontiguousarray(woA.astype(bf)),
            "cosF": cosF,
            "sinF": sinF,
        })

    if "nc" not in _cache:
        _cache["nc"] = _build_nc()
    res = run_bass_kernel_spmd(_cache["nc"], in_maps, core_ids=list(range(NCORES)))
    _cache["last_results"] = res

    y = np.zeros((TOK, D), dtype=np.float32)
    for rm in res.results:
        y += rm["y"].astype(np.float32)
    return y.reshape(B, S, D)
